# revision 19
# baseline (speedup 1.0000x reference)
"""Trainium2 Bass kernel for ChemicalNet (per-species MLP / MoE routing).

Strategy
--------
Only atoms whose species is in {1, 6, 7, 8} produce output (others are 0),
and each such atom only needs ITS OWN species' 3-layer MLP.  The reference
runs all 4 expert networks on all atoms; we route on the host instead:

- host: map species -> expert index, collect per-expert atom index lists
- shard: 2 cores per expert, each core gets half of that expert's atoms
  (the per-core in_map carries that expert's weights, so the single SPMD
  program is expert-agnostic)
- host passes the gathered embedding columns TRANSPOSED ([128, n]) so the
  device needs no transposes: PE contracts over the partition axis directly
- device: L1 matmul+SiLU, L2 matmul (2-step K accum)+SiLU, L3 matmul -> [1,n]
- host scatters the compact per-core outputs back to the full [N, 1] output

Matmuls run in float32r (TensorEngine full rate, ~1e-4 relative precision;
plain fp32 needs 4 cycles/column).  A `use_f32r=False` escape hatch builds
the full-fp32 program.

Per-chunk (512 atoms) the two 128-row halves of the hidden layer land in one
[128, 1024] PSUM tile so a single ACTIVATE applies SiLU to both (the scalar
engine does not pipeline ACTIVATEs; fewer/bigger is faster).  That merge
needs a bias constant along the free axis; biases in this problem are
identically zero, which the host verifies -- nonzero-bias inputs take a
(slower) per-half ACTIVATE path with per-partition bias.

The layer-3 [1, F] matmul accumulates into a corner of the layer-2 PSUM
tile after its ACTIVATE has read it (WAR handled by Tile), so all 8 PSUM
banks go to the 4-deep [128, 1024] pipeline pool.

All shapes are compile-time constants derived from the actual input
(the Bass program is built fresh per call).
"""

import numpy as np

import concourse.bass as bass
import concourse.tile as tile
from concourse import bacc, mybir
from concourse.bass_utils import run_bass_kernel_spmd

N_CORES = 8
NSPECIES = 4
SPECIES_Z = np.array([1, 6, 7, 8], dtype=np.int32)
MAXIDX = 118
D = 128          # embedding dim
H = 256          # hidden dim
F = 512          # atom-chunk size (one PSUM bank of fp32)
FP = mybir.dt.float32
SILU = mybir.ActivationFunctionType.Silu


def _build_program(npad: int, zero_bias: bool, mmdt):
    """One SPMD program: a 3-layer per-expert MLP over `npad` atom columns."""
    nc = bacc.Bacc("TRN2", target_bir_lowering=False, debug=False,
                   num_devices=N_CORES)

    embT_d = nc.dram_tensor("embT", [D, npad], mmdt, kind="ExternalInput")
    w1_d = nc.dram_tensor("w1", [D, H], mmdt, kind="ExternalInput")
    w2_d = nc.dram_tensor("w2", [2, 128, H], mmdt, kind="ExternalInput")
    w3_d = nc.dram_tensor("w3", [128, 2], mmdt, kind="ExternalInput")
    if not zero_bias:
        b1_d = nc.dram_tensor("b1", [128, 2], FP, kind="ExternalInput")
        b2_d = nc.dram_tensor("b2", [128, 2], FP, kind="ExternalInput")
        b3_d = nc.dram_tensor("b3", [1, 1], FP, kind="ExternalInput")
    out_d = nc.dram_tensor("out", [1, npad], FP, kind="ExternalOutput")

    # ramped chunk sizes: small first chunks let the first ACTIVATEs start
    # while the bulk of the embedding is still streaming in
    sizes = []
    for s in (128, 256):
        if sum(sizes) + s <= npad:
            sizes.append(s)
    while npad - sum(sizes) > F:
        sizes.append(F)
    if npad - sum(sizes):
        sizes.append(npad - sum(sizes))
    chunks = []
    c0 = 0
    for s in sizes:
        chunks.append((c0, s))
        c0 += s
    nch = len(chunks)

    with tile.TileContext(nc) as tc:
        with (
            tc.tile_pool(name="singles", bufs=1) as singles,
            tc.tile_pool(name="emb", bufs=nch) as embp,
            tc.tile_pool(name="z1p", bufs=nch) as z1p,
            tc.tile_pool(name="z2p", bufs=nch) as z2p,
            tc.tile_pool(name="outp", bufs=3) as outp,
            tc.tile_pool(name="ps", bufs=4, space="PSUM") as psp,
        ):
            # preload the SiLU table set while input DMAs run
            warm_act = singles.tile([128, 1], FP)
            nc.vector.memset(warm_act[:], 0.0)
            nc.scalar.activation(warm_act[:], warm_act[:], SILU)

            # --- inputs split over the two HWDGE queues (sync + scalar,
            # each ~250 GB/s) so transfers run in parallel, ordered so the
            # first chunk's dependencies complete first: emb0 leads the
            # sync queue while w1 leads the scalar queue.  w2/w3 go on the
            # (slower) gpsimd SWDGE queue; they are not needed until L2. ---
            emb_ts = []
            for ci, (c0, f) in enumerate(chunks):
                emb_c = embp.tile([D, F], mmdt, tag="emb", name=f"emb{ci}")
                emb_ts.append(emb_c)

            w1_t = singles.tile([D, H], mmdt)
            nc.sync.dma_start(w1_t[:], w1_d[:])
            for ci, (c0, f) in enumerate(chunks):
                nc.sync.dma_start(emb_ts[ci][:, :f], embT_d[:, c0:c0 + f])
            w2_t = singles.tile([128, 2 * H], mmdt)
            for r in range(2):
                nc.gpsimd.dma_start(w2_t[:, r * H:(r + 1) * H], w2_d[r])
            w3_t = singles.tile([128, 2], mmdt)
            nc.gpsimd.dma_start(w3_t[:], w3_d[:])
            if not zero_bias:
                b1_t = singles.tile([128, 2], FP)
                nc.gpsimd.dma_start(b1_t[:], b1_d[:])
                b2_t = singles.tile([128, 2], FP)
                nc.gpsimd.dma_start(b2_t[:], b2_d[:])
                b3_t = singles.tile([1, 1], FP)
                nc.gpsimd.dma_start(b3_t[:], b3_d[:])

            def m_off(f):
                # matmul output must stay inside one 512-col PSUM bank:
                # pack the m1 half right after m0 only when both fit bank 0
                return f if 2 * f <= F else F

            def act_pair(z_t, ps_t, f, b_t):
                """SiLU both m-halves of a psum tile -> z SBUF.

                Zero-bias path: ONE ACTIVATE over [0, m_off+f) -- for
                off==F > f this also covers the unused gap columns, which
                is harmless and cheaper than a second ACTIVATE."""
                off = m_off(f)
                if zero_bias:
                    nc.scalar.activation(z_t[:, :off + f], ps_t[:, :off + f],
                                         SILU)
                else:
                    for m in range(2):
                        nc.scalar.activation(
                            z_t[:, m * off:m * off + f],
                            ps_t[:, m * off:m * off + f], SILU,
                            bias=b_t[:, m:m + 1])

            # Software-pipelined emission: L1 runs two chunks ahead of L2,
            # L3 one behind, so the scalar engine's in-order ACTIVATE queue
            # never head-of-line blocks (ACT1(c+2) sits between ACT2(c) and
            # ACT2(c+1)) and the PE always has independent matmuls queued.
            z1s, z2s, ps2s = {}, {}, {}

            def emit_l1(ci):
                c0, f = chunks[ci]
                ps1 = psp.tile([128, 2 * F], FP, tag="ps", name=f"ps1_{ci}")
                off = m_off(f)
                for m in range(2):
                    nc.tensor.matmul(ps1[:, m * off:m * off + f],
                                     w1_t[:, m * 128:(m + 1) * 128],
                                     emb_ts[ci][:, :f], start=True, stop=True)
                z1 = z1p.tile([128, 2 * F], mmdt, tag="z1", name=f"z1_{ci}")
                act_pair(z1, ps1, f, None if zero_bias else b1_t)
                z1s[ci] = z1

            def emit_l2(ci):
                c0, f = chunks[ci]
                z1 = z1s[ci]
                off = m_off(f)
                ps2 = psp.tile([128, 2 * F], FP, tag="ps", name=f"ps2_{ci}")
                for m in range(2):
                    nc.tensor.matmul(ps2[:, m * off:m * off + f],
                                     w2_t[:, m * 128:m * 128 + 128],
                                     z1[:, :f], start=True, stop=False)
                    nc.tensor.matmul(ps2[:, m * off:m * off + f],
                                     w2_t[:, H + m * 128:H + m * 128 + 128],
                                     z1[:, off:off + f], start=False, stop=True)
                z2 = z2p.tile([128, 2 * F], mmdt, tag="z2", name=f"z2_{ci}")
                act_pair(z2, ps2, f, None if zero_bias else b2_t)
                z2s[ci], ps2s[ci] = z2, ps2

            def emit_l3(ci):
                c0, f = chunks[ci]
                z2 = z2s[ci]
                # L3 accumulates into a corner of ps2 after its ACT read
                # (WAR handled by Tile) -- no extra PSUM bank needed.
                ps3 = ps2s[ci][0:1, 0:f]
                off = m_off(f)
                nc.tensor.matmul(ps3, w3_t[:, 0:1], z2[:, :f],
                                 start=True, stop=False)
                nc.tensor.matmul(ps3, w3_t[:, 1:2], z2[:, off:off + f],
                                 start=False, stop=True)
                out_t = outp.tile([1, F], FP, tag="out", name=f"out_{ci}")
                if zero_bias:
                    nc.vector.tensor_copy(out_t[:, :f], ps3)
                else:
                    nc.vector.tensor_scalar_add(out_t[:, :f], ps3,
                                                b3_t[0:1, 0:1])
                nc.gpsimd.dma_start(out_d[:, c0:c0 + f], out_t[:, :f])

            depth = min(4, nch)
            for ci in range(depth):
                emit_l1(ci)
            for ci in range(nch):
                emit_l2(ci)
                if ci + depth < nch:
                    emit_l1(ci + depth)
                if ci >= 1:
                    emit_l3(ci - 1)
            emit_l3(nch - 1)

    nc.compile()
    return nc


def _route(species: np.ndarray):
    """species values -> expert idx (-1 unknown); per-core row assignments."""
    conv = np.full(MAXIDX + 2, -1, dtype=np.int32)
    conv[SPECIES_Z] = np.arange(NSPECIES, dtype=np.int32)
    idx = conv[species]
    core_rows = []
    for s in range(NSPECIES):
        rows = np.flatnonzero(idx == s)
        h = (len(rows) + 1) // 2
        core_rows.append(rows[:h])
        core_rows.append(rows[h:])
    return core_rows


def _run(inputs: dict, trace: bool = False, use_f32r: bool = True,
         use_bf16: bool = False):
    species = inputs["species"]
    embedding = np.ascontiguousarray(inputs["embedding"], dtype=np.float32)
    n_atoms = species.shape[0]
    out_full = np.zeros((n_atoms, 1), dtype=np.float32)

    core_rows = _route(np.asarray(species))
    nmax = max(len(r) for r in core_rows)
    if nmax == 0:
        return out_full, None
    npad = -(-nmax // 4) * 4

    zero_bias = all(
        not np.any(np.asarray(inputs[k])) for k in ("b1", "b2", "b3"))
    if use_bf16:
        mmdt = mybir.dt.bfloat16
    elif use_f32r:
        mmdt = mybir.dt.float32r
    else:
        mmdt = FP
    np_mm = mybir.dt.np(mmdt)
    nc = _build_program(npad, zero_bias, mmdt)

    in_maps = []
    for c in range(N_CORES):
        s = c // 2
        rows = core_rows[c]
        embT = np.zeros((D, npad), dtype=np_mm)
        if len(rows):
            embT[:, :len(rows)] = embedding[rows].T.astype(np_mm)
        im = {
            "embT": embT,
            "w1": np.ascontiguousarray(
                np.asarray(inputs["W1"][s], dtype=np.float32).astype(np_mm)),
            "w2": np.ascontiguousarray(np.asarray(
                inputs["W2"][s], dtype=np.float32).reshape(2, 128, H).astype(np_mm)),
            "w3": np.ascontiguousarray(np.asarray(
                inputs["W3"][s], dtype=np.float32).reshape(2, 128).T.astype(np_mm)),
        }
        if not zero_bias:
            im["b1"] = np.ascontiguousarray(
                np.asarray(inputs["b1"][s], dtype=np.float32).reshape(2, 128).T)
            im["b2"] = np.ascontiguousarray(
                np.asarray(inputs["b2"][s], dtype=np.float32).reshape(2, 128).T)
            im["b3"] = np.asarray(inputs["b3"][s], dtype=np.float32).reshape(1, 1)
        in_maps.append(im)

    res = run_bass_kernel_spmd(nc, in_maps, core_ids=list(range(N_CORES)),
                               trace=trace)
    for c in range(N_CORES):
        rows = core_rows[c]
        if len(rows):
            out_full[rows, 0] = res.results[c]["out"][0, :len(rows)]
    return out_full, res


def kernel(**inputs) -> np.ndarray:
    out, _ = _run(inputs, trace=False)
    return out


# revision 20
# speedup vs baseline: 1.0067x; 1.0067x over previous
"""Trainium2 Bass kernel for ChemicalNet (per-species MLP / MoE routing).

Strategy
--------
Only atoms whose species is in {1, 6, 7, 8} produce output (others are 0),
and each such atom only needs ITS OWN species' 3-layer MLP.  The reference
runs all 4 expert networks on all atoms; we route on the host instead:

- host: map species -> expert index, collect per-expert atom index lists
- shard: 2 cores per expert, each core gets half of that expert's atoms
  (the per-core in_map carries that expert's weights, so the single SPMD
  program is expert-agnostic)
- host passes the gathered embedding columns TRANSPOSED ([128, n]) so the
  device needs no transposes: PE contracts over the partition axis directly
- device: L1 matmul+SiLU, L2 matmul (2-step K accum)+SiLU, L3 matmul -> [1,n]
- host scatters the compact per-core outputs back to the full [N, 1] output

Matmuls run in float32r (TensorEngine full rate, ~1e-4 relative precision;
plain fp32 needs 4 cycles/column).  A `use_f32r=False` escape hatch builds
the full-fp32 program.

Per-chunk (512 atoms) the two 128-row halves of the hidden layer land in one
[128, 1024] PSUM tile so a single ACTIVATE applies SiLU to both (the scalar
engine does not pipeline ACTIVATEs; fewer/bigger is faster).  That merge
needs a bias constant along the free axis; biases in this problem are
identically zero, which the host verifies -- nonzero-bias inputs take a
(slower) per-half ACTIVATE path with per-partition bias.

The layer-3 [1, F] matmul accumulates into a corner of the layer-2 PSUM
tile after its ACTIVATE has read it (WAR handled by Tile), so all 8 PSUM
banks go to the 4-deep [128, 1024] pipeline pool.

All shapes are compile-time constants derived from the actual input
(the Bass program is built fresh per call).
"""

import numpy as np

import concourse.bass as bass
import concourse.tile as tile
from concourse import bacc, mybir
from concourse.bass_utils import run_bass_kernel_spmd

N_CORES = 8
NSPECIES = 4
SPECIES_Z = np.array([1, 6, 7, 8], dtype=np.int32)
MAXIDX = 118
D = 128          # embedding dim
H = 256          # hidden dim
F = 512          # atom-chunk size (one PSUM bank of fp32)
FP = mybir.dt.float32
SILU = mybir.ActivationFunctionType.Silu


def _build_program(npad: int, zero_bias: bool, mmdt):
    """One SPMD program: a 3-layer per-expert MLP over `npad` atom columns."""
    nc = bacc.Bacc("TRN2", target_bir_lowering=False, debug=False,
                   num_devices=N_CORES)

    embT_d = nc.dram_tensor("embT", [D, npad], mmdt, kind="ExternalInput")
    w1_d = nc.dram_tensor("w1", [D, H], mmdt, kind="ExternalInput")
    w2_d = nc.dram_tensor("w2", [2, 128, H], mmdt, kind="ExternalInput")
    w3_d = nc.dram_tensor("w3", [128, 2], mmdt, kind="ExternalInput")
    if not zero_bias:
        b1_d = nc.dram_tensor("b1", [128, 2], FP, kind="ExternalInput")
        b2_d = nc.dram_tensor("b2", [128, 2], FP, kind="ExternalInput")
        b3_d = nc.dram_tensor("b3", [1, 1], FP, kind="ExternalInput")
    out_d = nc.dram_tensor("out", [1, npad], FP, kind="ExternalOutput")

    # ramped chunk sizes: small first chunks let the first ACTIVATEs start
    # while the bulk of the embedding is still streaming in
    sizes = []
    for s in (128, 256):
        if sum(sizes) + s <= npad:
            sizes.append(s)
    while npad - sum(sizes) > F:
        sizes.append(F)
    if npad - sum(sizes):
        sizes.append(npad - sum(sizes))
    chunks = []
    c0 = 0
    for s in sizes:
        chunks.append((c0, s))
        c0 += s
    nch = len(chunks)

    with tile.TileContext(nc) as tc:
        with (
            tc.tile_pool(name="singles", bufs=1) as singles,
            tc.tile_pool(name="emb", bufs=nch) as embp,
            tc.tile_pool(name="z1p", bufs=nch) as z1p,
            tc.tile_pool(name="z2p", bufs=nch) as z2p,
            tc.tile_pool(name="outp", bufs=3) as outp,
            tc.tile_pool(name="ps", bufs=4, space="PSUM") as psp,
        ):
            # preload the SiLU table set while input DMAs run
            warm_act = singles.tile([128, 1], FP)
            nc.vector.memset(warm_act[:], 0.0)
            nc.scalar.activation(warm_act[:], warm_act[:], SILU)

            # --- inputs split over the two HWDGE queues (sync + scalar,
            # each ~250 GB/s) so transfers run in parallel, ordered so the
            # first chunk's dependencies complete first: emb0 leads the
            # sync queue while w1 leads the scalar queue.  w2/w3 go on the
            # (slower) gpsimd SWDGE queue; they are not needed until L2. ---
            emb_ts = []
            for ci, (c0, f) in enumerate(chunks):
                emb_c = embp.tile([D, F], mmdt, tag="emb", name=f"emb{ci}")
                emb_ts.append(emb_c)

            w1_t = singles.tile([D, H], mmdt)
            nc.sync.dma_start(w1_t[:], w1_d[:])
            for ci, (c0, f) in enumerate(chunks):
                nc.sync.dma_start(emb_ts[ci][:, :f], embT_d[:, c0:c0 + f])
            w2_t = singles.tile([128, 2 * H], mmdt)
            for r in range(2):
                nc.gpsimd.dma_start(w2_t[:, r * H:(r + 1) * H], w2_d[r])
            w3_t = singles.tile([128, 2], mmdt)
            nc.gpsimd.dma_start(w3_t[:], w3_d[:])
            if not zero_bias:
                b1_t = singles.tile([128, 2], FP)
                nc.gpsimd.dma_start(b1_t[:], b1_d[:])
                b2_t = singles.tile([128, 2], FP)
                nc.gpsimd.dma_start(b2_t[:], b2_d[:])
                b3_t = singles.tile([1, 1], FP)
                nc.gpsimd.dma_start(b3_t[:], b3_d[:])

            def m_off(f):
                # matmul output must stay inside one 512-col PSUM bank:
                # pack the m1 half right after m0 only when both fit bank 0
                return f if 2 * f <= F else F

            def act_pair(z_t, ps_t, f, b_t):
                """SiLU both m-halves of a psum tile -> z SBUF.

                Zero-bias path: ONE ACTIVATE over [0, m_off+f) -- for
                off==F > f this also covers the unused gap columns, which
                is harmless and cheaper than a second ACTIVATE."""
                off = m_off(f)
                if zero_bias:
                    nc.scalar.activation(z_t[:, :off + f], ps_t[:, :off + f],
                                         SILU)
                else:
                    for m in range(2):
                        nc.scalar.activation(
                            z_t[:, m * off:m * off + f],
                            ps_t[:, m * off:m * off + f], SILU,
                            bias=b_t[:, m:m + 1])

            # Software-pipelined emission: L1 runs two chunks ahead of L2,
            # L3 one behind, so the scalar engine's in-order ACTIVATE queue
            # never head-of-line blocks (ACT1(c+2) sits between ACT2(c) and
            # ACT2(c+1)) and the PE always has independent matmuls queued.
            z1s, z2s, ps2s = {}, {}, {}

            def emit_l1(ci):
                c0, f = chunks[ci]
                ps1 = psp.tile([128, 2 * F], FP, tag="ps", name=f"ps1_{ci}")
                off = m_off(f)
                for m in range(2):
                    nc.tensor.matmul(ps1[:, m * off:m * off + f],
                                     w1_t[:, m * 128:(m + 1) * 128],
                                     emb_ts[ci][:, :f], start=True, stop=True)
                z1 = z1p.tile([128, 2 * F], mmdt, tag="z1", name=f"z1_{ci}")
                act_pair(z1, ps1, f, None if zero_bias else b1_t)
                z1s[ci] = z1

            def emit_l2(ci):
                c0, f = chunks[ci]
                z1 = z1s[ci]
                off = m_off(f)
                ps2 = psp.tile([128, 2 * F], FP, tag="ps", name=f"ps2_{ci}")
                for m in range(2):
                    nc.tensor.matmul(ps2[:, m * off:m * off + f],
                                     w2_t[:, m * 128:m * 128 + 128],
                                     z1[:, :f], start=True, stop=False)
                    nc.tensor.matmul(ps2[:, m * off:m * off + f],
                                     w2_t[:, H + m * 128:H + m * 128 + 128],
                                     z1[:, off:off + f], start=False, stop=True)
                z2 = z2p.tile([128, 2 * F], mmdt, tag="z2", name=f"z2_{ci}")
                act_pair(z2, ps2, f, None if zero_bias else b2_t)
                z2s[ci], ps2s[ci] = z2, ps2

            def emit_l3(ci):
                c0, f = chunks[ci]
                z2 = z2s[ci]
                # L3 accumulates into a corner of ps2 after its ACT read
                # (WAR handled by Tile) -- no extra PSUM bank needed.
                ps3 = ps2s[ci][0:1, 0:f]
                off = m_off(f)
                nc.tensor.matmul(ps3, w3_t[:, 0:1], z2[:, :f],
                                 start=True, stop=False)
                nc.tensor.matmul(ps3, w3_t[:, 1:2], z2[:, off:off + f],
                                 start=False, stop=True)
                out_t = outp.tile([1, F], FP, tag="out", name=f"out_{ci}")
                if zero_bias:
                    nc.vector.tensor_copy(out_t[:, :f], ps3)
                else:
                    nc.vector.tensor_scalar_add(out_t[:, :f], ps3,
                                                b3_t[0:1, 0:1])
                nc.gpsimd.dma_start(out_d[:, c0:c0 + f], out_t[:, :f])

            depth = min(3, nch)
            for ci in range(depth):
                emit_l1(ci)
            for ci in range(nch):
                emit_l2(ci)
                if ci + depth < nch:
                    emit_l1(ci + depth)
                if ci >= 1:
                    emit_l3(ci - 1)
            emit_l3(nch - 1)

    nc.compile()
    return nc


def _route(species: np.ndarray):
    """species values -> expert idx (-1 unknown); per-core row assignments."""
    conv = np.full(MAXIDX + 2, -1, dtype=np.int32)
    conv[SPECIES_Z] = np.arange(NSPECIES, dtype=np.int32)
    idx = conv[species]
    core_rows = []
    for s in range(NSPECIES):
        rows = np.flatnonzero(idx == s)
        h = (len(rows) + 1) // 2
        core_rows.append(rows[:h])
        core_rows.append(rows[h:])
    return core_rows


def _run(inputs: dict, trace: bool = False, use_f32r: bool = True,
         use_bf16: bool = False):
    species = inputs["species"]
    embedding = np.ascontiguousarray(inputs["embedding"], dtype=np.float32)
    n_atoms = species.shape[0]
    out_full = np.zeros((n_atoms, 1), dtype=np.float32)

    core_rows = _route(np.asarray(species))
    nmax = max(len(r) for r in core_rows)
    if nmax == 0:
        return out_full, None
    npad = -(-nmax // 4) * 4

    zero_bias = all(
        not np.any(np.asarray(inputs[k])) for k in ("b1", "b2", "b3"))
    if use_bf16:
        mmdt = mybir.dt.bfloat16
    elif use_f32r:
        mmdt = mybir.dt.float32r
    else:
        mmdt = FP
    np_mm = mybir.dt.np(mmdt)
    nc = _build_program(npad, zero_bias, mmdt)

    in_maps = []
    for c in range(N_CORES):
        s = c // 2
        rows = core_rows[c]
        embT = np.zeros((D, npad), dtype=np_mm)
        if len(rows):
            embT[:, :len(rows)] = embedding[rows].T.astype(np_mm)
        im = {
            "embT": embT,
            "w1": np.ascontiguousarray(
                np.asarray(inputs["W1"][s], dtype=np.float32).astype(np_mm)),
            "w2": np.ascontiguousarray(np.asarray(
                inputs["W2"][s], dtype=np.float32).reshape(2, 128, H).astype(np_mm)),
            "w3": np.ascontiguousarray(np.asarray(
                inputs["W3"][s], dtype=np.float32).reshape(2, 128).T.astype(np_mm)),
        }
        if not zero_bias:
            im["b1"] = np.ascontiguousarray(
                np.asarray(inputs["b1"][s], dtype=np.float32).reshape(2, 128).T)
            im["b2"] = np.ascontiguousarray(
                np.asarray(inputs["b2"][s], dtype=np.float32).reshape(2, 128).T)
            im["b3"] = np.asarray(inputs["b3"][s], dtype=np.float32).reshape(1, 1)
        in_maps.append(im)

    res = run_bass_kernel_spmd(nc, in_maps, core_ids=list(range(N_CORES)),
                               trace=trace)
    for c in range(N_CORES):
        rows = core_rows[c]
        if len(rows):
            out_full[rows, 0] = res.results[c]["out"][0, :len(rows)]
    return out_full, res


def kernel(**inputs) -> np.ndarray:
    out, _ = _run(inputs, trace=False)
    return out
